# revision 18
# baseline (speedup 1.0000x reference)
"""DIN-attention kernel for Trainium2, 8-core SPMD.

Reference computation (per batch b, seq pos l, x = item_seq[b, l]):
    mlp_in = [tgt, x, x-tgt, x*tgt]           (4D = 512)
    h      = relu(mlp_in @ W1 + b1)           (2D = 256)
    score  = h @ W2 + b2                      (1)
    out_b  = sum_l score[l] * x[l] * (l < seq_len[b])

Algebraic restructure (W1 = [A; B; C; Dm] in 128-row blocks):
    z   = x @ (B + C) + (x*tgt) @ Dm + c_b,   c_b = tgt_b @ (A - C) + b1
    out = sum_{l < n_b} (W2.T relu(z) + b2) * x[l]

Device strategy (per core):
  - Batches sorted by seq_len descending; slot s holds global ranks
    [8s, 8s+8), one per core, padded to a shared per-slot length L_s
    (max over the 8, rounded even).  Zero-padded columns contribute
    exactly 0 to the output, so padding is safe, and all 8 cores run an
    identical (SPMD) program while loading only ~half the dense bytes.
  - Tokens packed host-side into a transposed (128=D, T) fp32 array per
    core; compute in the "hidden-on-partitions" layout:
      zT (128=hid_half, n) = Wbc_h.T @ X + Wd_h.T @ Y + Cwin_h.T @ IND
    with Y = X * tgt_col (per-slot, gpsimd) and IND a host-built 0/1
    (32, T) slot-window indicator; Cwin packs the c_b bias rows.
  - relu on ScalarE, then score broadcast to all 128 partitions in one
    PSUM accumulation: P = W2rep0.T @ r0 + W2rep1.T @ r1 + b2 * ones
    (W2rep[k, m] = W2[k] for every m, so every output row = score row).
  - Final per-slot reduce: fused DVE tensor_tensor_reduce
    acc[:, s] = sum_cols (X * P), chained across 512-tiles via initial.
  - Matmuls run in float32r (fp32 storage, single-pass PE streaming).
"""

import sys

import numpy as np

for _p in ("/opt/trn_rl_repo",):
    if _p not in sys.path:
        sys.path.insert(0, _p)

import concourse.bacc as bacc
import concourse.bass as bass
import concourse.tile as tile
from concourse import mybir
from concourse.bass_utils import run_bass_kernel_spmd

assert bass  # re-exported for callers

B_FULL = 2048
L_FULL = 200
D = 128
N_CORES = 8
HID = 256  # 2D
TILE_N = 512  # fp32 PSUM bank columns
CHUNK_TARGET = 8192  # tokens per streamed chunk (slot-aligned)
F32 = mybir.dt.float32
F32R = mybir.dt.float32r
BF16 = mybir.dt.bfloat16

HOST_Y_BF16 = True  # ship Y = X*tgt as a host-packed bf16 array
RELU_BF16 = False  # bf16 ACT output is broken on TRN2 HW (probe E); use f32r
REDUCE_MODE = "stt"  # "ttr" | "stt" | "ttred"  (final per-slot reduce impl)


def _plan(seq_len):
    """Slot plan shared by all cores (SPMD: identical program)."""
    n = np.clip(np.asarray(seq_len).astype(np.int64), 0, L_FULL)
    order = np.argsort(-n, kind="stable")  # descending
    n_sorted = n[order]
    slot_lens = []
    for s in range(B_FULL // N_CORES):
        m = int(n_sorted[N_CORES * s])  # max of ranks [8s, 8s+8)
        if m <= 0:
            break
        slot_lens.append(m + (m & 1))  # round up to even
    S = len(slot_lens)
    offs = np.zeros(S + 1, dtype=np.int64)
    offs[1:] = np.cumsum(slot_lens)
    T = int(offs[-1])

    # chunks: contiguous slot ranges with <= CHUNK_TARGET tokens
    chunks = []  # (slot_a, slot_b, tok_off, tok_len)
    sa = 0
    while sa < S:
        sb = sa
        while sb < S and offs[sb + 1] - offs[sa] <= CHUNK_TARGET:
            sb += 1
        if sb == sa:
            sb = sa + 1
        chunks.append((sa, sb, int(offs[sa]), int(offs[sb] - offs[sa])))
        sa = sb
    return n, order, slot_lens, offs, T, chunks


def _build_program(slot_lens, offs, T, chunks):
    S = len(slot_lens)
    NW = (S + 31) // 32  # 32-slot bias windows
    nc = bacc.Bacc("TRN2", target_bir_lowering=False, debug=False)

    RDT = BF16 if RELU_BF16 else F32R
    YDT = BF16 if HOST_Y_BF16 else F32

    xt_d = nc.dram_tensor("xt", [D, T], F32R, kind="ExternalInput")
    ind_d = nc.dram_tensor("ind", [32, T], BF16, kind="ExternalInput")
    if HOST_Y_BF16:
        yb_d = nc.dram_tensor("yb", [D, T], BF16, kind="ExternalInput")
    else:
        tgt_d = nc.dram_tensor("tgt", [D, S], F32, kind="ExternalInput")
    cbw_d = nc.dram_tensor("cbw", [32, NW * HID], BF16, kind="ExternalInput")
    wbc_d = nc.dram_tensor("wbc", [D, HID], F32R, kind="ExternalInput")
    wd_d = nc.dram_tensor("wd", [D, HID], YDT, kind="ExternalInput")
    w2r_d = nc.dram_tensor("w2r", [D, HID], RDT, kind="ExternalInput")
    b2v_d = nc.dram_tensor("b2v", [1, D], BF16, kind="ExternalInput")
    out_d = nc.dram_tensor("out_t", [D, 256], F32, kind="ExternalOutput")

    cmax = max(c[3] for c in chunks)

    with tile.TileContext(nc) as tc:
        with (
            tc.tile_pool(name="const", bufs=1) as cpool,
            tc.tile_pool(name="xst", bufs=2) as xpool,
            tc.tile_pool(name="yst", bufs=2) as ypool,
            tc.tile_pool(name="ist", bufs=2) as ipool,
            tc.tile_pool(name="rst", bufs=2) as rpool,
            tc.tile_pool(name="dst", bufs=2) as dpool,
            tc.tile_pool(name="ps", bufs=2, space="PSUM") as pspool,
        ):
            wbc = cpool.tile([D, HID], F32R, tag="wbc")
            wd = cpool.tile([D, HID], YDT, tag="wd")
            w2r = cpool.tile([D, HID], RDT, tag="w2r")
            cbw = cpool.tile([32, NW * HID], BF16, tag="cbw")
            b2v = cpool.tile([1, D], BF16, tag="b2v")
            ones = cpool.tile([1, TILE_N], BF16, tag="ones")
            acc = cpool.tile([D, 256], F32, tag="acc")
            aux = cpool.tile([D, 2], F32, tag="aux")

            nc.sync.dma_start(out=wbc[:], in_=wbc_d[:])
            nc.sync.dma_start(out=wd[:], in_=wd_d[:])
            nc.sync.dma_start(out=w2r[:], in_=w2r_d[:])
            if not HOST_Y_BF16:
                tgt = cpool.tile([D, S], F32, tag="tgt")
                nc.sync.dma_start(out=tgt[:], in_=tgt_d[:])
            nc.sync.dma_start(out=cbw[:], in_=cbw_d[:])
            nc.sync.dma_start(out=b2v[:], in_=b2v_d[:])
            nc.vector.memset(ones[:], 1.0)
            nc.vector.memset(acc[:], 0.0)

            for sa, sb, toff, tlen in chunks:
                x = xpool.tile([D, cmax], F32R, tag="x")
                y = ypool.tile([D, cmax], YDT, tag="y")
                indt = ipool.tile([32, cmax], BF16, tag="ind")
                nc.sync.dma_start(out=x[:, :tlen], in_=xt_d[:, toff : toff + tlen])
                nc.sync.dma_start(out=indt[:, :tlen], in_=ind_d[:, toff : toff + tlen])

                if HOST_Y_BF16:
                    nc.sync.dma_start(
                        out=y[:, :tlen], in_=yb_d[:, toff : toff + tlen]
                    )
                else:
                    # Y = X * tgt_b  (per-slot columns, per-partition scalar)
                    for s in range(sa, sb):
                        a = int(offs[s] - toff)
                        b = int(offs[s + 1] - toff)
                        nc.gpsimd.tensor_scalar_mul(
                            y[:, a:b], x[:, a:b].bitcast(F32), tgt[:, s : s + 1]
                        )

                ntiles = (tlen + TILE_N - 1) // TILE_N
                for j in range(ntiles):
                    c0 = j * TILE_N
                    c1 = min(tlen, c0 + TILE_N)
                    n = c1 - c0
                    # slot segments covered by this tile (chunk-local cols)
                    segs = []
                    for s in range(sa, sb):
                        a = max(int(offs[s] - toff), c0)
                        b = min(int(offs[s + 1] - toff), c1)
                        if a < b:
                            segs.append((s, a, b))

                    zz = []
                    for h in (0, 1):
                        z = pspool.tile([D, TILE_N], F32, tag=f"z{h}")
                        hs = slice(h * D, h * D + D)
                        nc.tensor.matmul(
                            z[:, :n],
                            wbc[:, hs],
                            x[:, c0:c1],
                            start=True,
                            stop=False,
                        )
                        if HOST_Y_BF16:
                            nc.tensor.matmul(
                                z[:, :n],
                                wd[:, hs],
                                y[:, c0:c1],
                                start=False,
                                stop=False,
                            )
                        else:
                            nc.tensor.matmul(
                                z[:, :n],
                                wd[:, hs].bitcast(F32R),
                                y[:, c0:c1].bitcast(F32R),
                                start=False,
                                stop=False,
                            )
                        # per-slot bias via 32-slot window indicator matmul
                        wins = {}
                        for s, a, b in segs:
                            w = s // 32
                            if w in wins:
                                lo, hi = wins[w]
                                wins[w] = (min(lo, a), max(hi, b))
                            else:
                                wins[w] = (a, b)
                        witems = sorted(wins.items())
                        for wi, (w, (a, b)) in enumerate(witems):
                            nc.tensor.matmul(
                                z[:, a - c0 : b - c0],
                                cbw[
                                    :, w * HID + h * D : w * HID + h * D + D
                                ],
                                indt[:, a:b],
                                start=False,
                                stop=(wi == len(witems) - 1),
                            )
                        zz.append(z)

                    r0 = rpool.tile([D, TILE_N], RDT, tag="r0")
                    r1 = rpool.tile([D, TILE_N], RDT, tag="r1")
                    nc.scalar.activation(
                        r0[:, :n], zz[0][:, :n], mybir.ActivationFunctionType.Relu
                    )
                    nc.scalar.activation(
                        r1[:, :n], zz[1][:, :n], mybir.ActivationFunctionType.Relu
                    )

                    # P[:, t] = score(t) + b2 on every partition
                    pbc = pspool.tile([D, TILE_N], F32, tag="pbc")
                    if RELU_BF16:
                        w2r0, w2r1 = w2r[:, 0:D], w2r[:, D:HID]
                        rr0, rr1 = r0[:, :n], r1[:, :n]
                    else:
                        w2r0 = w2r[:, 0:D].bitcast(F32R)
                        w2r1 = w2r[:, D:HID].bitcast(F32R)
                        rr0 = r0[:, :n].bitcast(F32R)
                        rr1 = r1[:, :n].bitcast(F32R)
                    nc.tensor.matmul(pbc[:, :n], w2r0, rr0, start=True, stop=False)
                    nc.tensor.matmul(pbc[:, :n], w2r1, rr1, start=False, stop=False)
                    nc.tensor.matmul(
                        pbc[:, :n],
                        b2v[:],
                        ones[:, :n],
                        start=False,
                        stop=True,
                    )

                    dump = dpool.tile([D, TILE_N], F32, tag="dump")
                    if REDUCE_MODE == "ttr":
                        for s, a, b in segs:
                            first = a == int(offs[s] - toff)
                            nc.vector.tensor_tensor_reduce(
                                out=dump[:, a - c0 : b - c0],
                                in0=x[:, a:b].bitcast(F32),
                                in1=pbc[:, a - c0 : b - c0],
                                scale=1.0,
                                scalar=0.0 if first else acc[:, s : s + 1],
                                op0=mybir.AluOpType.mult,
                                op1=mybir.AluOpType.add,
                                accum_out=acc[:, s : s + 1],
                            )
                    elif REDUCE_MODE == "stt":
                        for s, a, b in segs:
                            first = a == int(offs[s] - toff)
                            tgt_col = (
                                acc[:, s : s + 1]
                                if first
                                else aux[:, 0:1]
                            )
                            nc.vector.scalar_tensor_tensor(
                                out=dump[:, a - c0 : b - c0],
                                in0=x[:, a:b].bitcast(F32),
                                scalar=1.0,
                                in1=pbc[:, a - c0 : b - c0],
                                op0=mybir.AluOpType.mult,
                                op1=mybir.AluOpType.mult,
                                accum_out=tgt_col,
                            )
                            if not first:
                                nc.vector.tensor_add(
                                    acc[:, s : s + 1],
                                    acc[:, s : s + 1],
                                    aux[:, 0:1],
                                )
                    else:  # "ttred"
                        nc.vector.tensor_tensor(
                            out=dump[:, :n],
                            in0=x[:, c0:c1].bitcast(F32),
                            in1=pbc[:, :n],
                            op=mybir.AluOpType.mult,
                        )
                        for s, a, b in segs:
                            first = a == int(offs[s] - toff)
                            tgt_col = (
                                acc[:, s : s + 1] if first else aux[:, 0:1]
                            )
                            nc.vector.tensor_reduce(
                                out=tgt_col,
                                in_=dump[:, a - c0 : b - c0],
                                axis=mybir.AxisListType.X,
                                op=mybir.AluOpType.add,
                            )
                            if not first:
                                nc.vector.tensor_add(
                                    acc[:, s : s + 1],
                                    acc[:, s : s + 1],
                                    aux[:, 0:1],
                                )

            nc.sync.dma_start(out=out_d[:], in_=acc[:])
    nc.compile()
    return nc


def _pack_core(item_seq, target, cmat, nvec, order, slot_lens, offs, T, core):
    S = len(slot_lens)
    NW = (S + 31) // 32
    x_nat = np.zeros((T, D), dtype=np.float32)
    y_nat = np.zeros((T, D), dtype=np.float32) if HOST_Y_BF16 else None
    from ml_dtypes import bfloat16

    ind = np.zeros((32, T), dtype=bfloat16)
    tgt = np.zeros((D, S), dtype=np.float32)
    cbw = np.zeros((32, NW * HID), dtype=bfloat16)
    for s in range(S):
        b = int(order[N_CORES * s + core])
        o = int(offs[s])
        nb = int(nvec[b])
        if nb > 0:
            x_nat[o : o + nb] = item_seq[b, :nb]
            if y_nat is not None:
                y_nat[o : o + nb] = item_seq[b, :nb] * target[b]
        ind[s % 32, o : o + slot_lens[s]] = 1.0
        tgt[:, s] = target[b]
        cbw[s % 32, (s // 32) * HID : (s // 32 + 1) * HID] = cmat[b]
    xt = np.ascontiguousarray(x_nat.T)
    m = {"xt": xt, "ind": ind, "cbw": cbw}
    if HOST_Y_BF16:
        from ml_dtypes import bfloat16

        m["yb"] = np.ascontiguousarray(y_nat.T).astype(bfloat16)
    else:
        m["tgt"] = tgt
    return m


def build_all(target, item_seq, seq_len, W1, b1, W2, b2):
    """Build (nc, in_maps, assemble) without running — used by kernel()
    and by test harnesses that want to run/profile the program."""
    target = np.asarray(target, dtype=np.float32)
    item_seq = np.asarray(item_seq, dtype=np.float32)
    W1 = np.asarray(W1, dtype=np.float32)
    b1 = np.asarray(b1, dtype=np.float32)
    W2 = np.asarray(W2, dtype=np.float32)
    b2 = np.asarray(b2, dtype=np.float32)

    nvec, order, slot_lens, offs, T, chunks = _plan(seq_len)
    S = len(slot_lens)

    W1a, W1b = W1[0:D], W1[D : 2 * D]
    W1c, W1d = W1[2 * D : 3 * D], W1[3 * D : 4 * D]
    wbc = np.ascontiguousarray(W1b + W1c)
    wd = np.ascontiguousarray(W1d)
    cmat = (target @ (W1a - W1c) + b1).astype(np.float32)  # (B, 256)
    w2r = np.empty((D, HID), dtype=np.float32)
    w2r[:, 0:D] = np.repeat(W2[0:D, 0:1], D, axis=1)  # [k, m] = W2[k]
    w2r[:, D:HID] = np.repeat(W2[D:HID, 0:1], D, axis=1)
    from ml_dtypes import bfloat16

    b2v = np.full((1, D), float(np.asarray(b2).reshape(-1)[0]), dtype=bfloat16)

    if HOST_Y_BF16 or RELU_BF16:
        from ml_dtypes import bfloat16
    if HOST_Y_BF16:
        wd = wd.astype(bfloat16)
    if RELU_BF16:
        w2r = w2r.astype(bfloat16)

    nc = _build_program(slot_lens, offs, T, chunks)

    shared = {"wbc": wbc, "wd": wd, "w2r": w2r, "b2v": b2v}
    in_maps = []
    for k in range(N_CORES):
        m = _pack_core(item_seq, target, cmat, nvec, order, slot_lens, offs, T, k)
        m.update(shared)
        in_maps.append(m)

    def assemble(results):
        out = np.zeros((B_FULL, D), dtype=np.float32)
        for k in range(N_CORES):
            ot = np.asarray(results[k]["out_t"])  # (128, 256)
            for s in range(S):
                out[int(order[N_CORES * s + k])] = ot[:, s]
        return out

    return nc, in_maps, assemble


def kernel(target, item_seq, seq_len, W1, b1, W2, b2):
    nc, in_maps, assemble = build_all(target, item_seq, seq_len, W1, b1, W2, b2)
    res = run_bass_kernel_spmd(nc, in_maps, list(range(N_CORES)))
    results = res.results if hasattr(res, "results") else res
    return assemble(results)
